# revision 80
# baseline (speedup 1.0000x reference)
"""Multi-head self-attention (B=8, S=1024, E=768, H=12, D=64) on 8 NeuronCores.

Sharding: data-parallel over batch — one batch element per core, weights
replicated, no collectives.  Measured ~216us HW exec (vs 263us f32r
baseline), rel-rms err 5.8e-3 vs the fp32 reference.

The kernel is jointly limited by the PE (~179us of matmul stream; bf16
runs 2 cols/cycle, fp8 is numerically infeasible here) and ACT (~107us of
exp; (N+352)/1.2ns per activation).  Everything else is scheduled around
keeping both dense:

  1. x arrives as four 0.75MB DMAs alternating the sync/scalar issue
     queues (each hwdge engine owns one ~80-130GB/s queue; >=1MB batches
     are fastest).  The PE transposes x per s-tile (XBAR DMA transpose
     into the strided xT layout measured ~8x slower); evacuation copies
     alternate DVE / ACT-Copy so no single engine paces the pipeline.
  2. V_ext[s, ktile, head, 128] = [V_h+bv | ones] (even head) or
     [ones | V_h+bv] (odd head); bv folded exactly (softmax rows sum to
     1).  The ones columns make each attnV matmul also produce the
     softmax denominators for free (M doesn't affect PE time).
  3. Attention runs as 12 software-pipelined units (pair m, q-half q2),
     ordered (0,0),(0,1),...,(2,1),(3,0),(4,0),(5,0),(3,1),(4,1),(5,1) so
     the q2=0 normalization + first output tiles overlap the last exps.
     PSUM holds only 2 of the 8 score tiles per unit, so PE emission
     interleaves at quarter-unit granularity: scores(i,c) / attnV(i-1)
     2-ktile chunks / ~3-matmul filler micro-ops (projection chain
     halves, V-pass tiles, output tiles, normalization broadcasts).
  4. Softmax normalization is deferred and batched per q-half: one sums
     row per head is staged (aligned DVE row copy, PSUM->SBUF) and DMA'd
     into a [12, 512] collect tile; ONE DVE reciprocal per batch
     (reciprocal cost is per-free-element, so batching 6 units is ~6x
     cheaper; reciprocal_approx_fast hits "ISA wrong length" in this
     walrus build), then a K=12 selector matmul broadcasts the
     reciprocals across partitions and a DVE multiply normalizes concatT
     in place.
  5. out = concatT.T @ Wo + bo; output stores stripe across both hwdge
     queues.  Weight casts f32->bf16 run on DVE when on the critical path
     and on the otherwise-idle gpsimd during the attention phase.

Numerics: bf16 operands everywhere, fp32 PSUM accumulation; exp folds the
1/sqrt(D)=1/8 scale (scores ~N(0,1), no max subtraction needed).

Hardware workarounds: this walrus build caps sync waits per instruction
(_split_excess_waits hoists the surplus onto standalone EVSEM ops); DVE
ops need 32-aligned partition bases and must not read PSUM at a partition
base different from their output's; custom DVE ops and
--enable-ldw-opt=true are broken in this build and avoided.
"""
import sys
sys.path.insert(0, "/opt/trn_rl_repo")
from contextlib import ExitStack

import numpy as np

import concourse.bass as bass
import concourse.tile as tile
from concourse import mybir
from concourse.bass_utils import run_bass_kernel_spmd
from concourse.masks import make_identity
from concourse.vector_clock import ScopedClock




def _split_drain_and_barrier(self, tick_clock, wait_clock):
    """TileContext tail with the final drain's waits split one-per-instruction."""
    drain_inst = self.nc.sync.drain()
    wait_clock.add_sem_waits(
        drain_inst.ins, ScopedClock({None: tick_clock.global_clock})
    )
    si = drain_inst.ins.sync_info
    waits = list(si.on_wait) if si is not None and si.on_wait else []
    if len(waits) > 1:
        si.on_wait = []
        by_num = {h.num: h for h in self.sems.allocated().values()}
        for w in waits:
            self.nc.sync.wait_ge(by_num[w.id], w.wait_value)
    self.nc.all_engine_barrier()
    popped = self.nc._tile_sem_poison_stack.pop()
    assert popped is self._sem_poison
    self.nc.clear_and_free_semaphores(list(self.sems.allocated().values()))
    self.nc.all_engine_barrier()


tile.TileContext._drain_and_barrier = _split_drain_and_barrier


def _split_excess_waits(nc):
    """Hoist excess per-instruction sync waits into standalone EVSEM waits."""
    counter = 0
    for f in nc.m.functions:
        for bb in f.blocks:
            insts = bb.instructions
            out = []
            for inst in insts:
                si = inst.sync_info
                cap = 2 if isinstance(inst, mybir.InstEventSemaphore) else 1
                if si is not None and si.on_wait and len(si.on_wait) > cap:
                    waits = list(si.on_wait)
                    for w in waits[cap:]:
                        counter += 1
                        ev = mybir.InstEventSemaphore(name=f"I-wsplit-{counter}")
                        ev.engine = inst.engine
                        ev.sync_info = mybir.SyncInfo(on_wait=[w], on_update=[])
                        out.append(ev)
                    si.on_wait = waits[:cap]
                out.append(inst)
            if len(out) != len(insts):
                insts[:] = out
    return counter

P = 128
S = 1024
E = 768
H = 12
D = 64
KT = E // P        # 6 e-tiles
ST = S // P        # 8 s-tiles
NPAIR = H // 2     # 6 head pairs
QTILE = 512
NQ = S // QTILE    # 2 q-tiles
ESLICES = [(0, 512), (512, 256)]

f32 = mybir.dt.float32
f32r = mybir.dt.float32r
bf16 = mybir.dt.bfloat16
EXP = mybir.ActivationFunctionType.Exp

_NC_CACHE = {}


def build(mm_dtype="bf16", e_dtype="bf16"):
    assert mm_dtype == "bf16" and e_dtype == "bf16"
    mdt = bf16
    edt = bf16
    nc = bass.Bass()
    x_d = nc.declare_dram_parameter("x", [S, E], f32, isOutput=False)
    Wq_d = nc.declare_dram_parameter("Wq", [E, E], f32, isOutput=False)
    Wk_d = nc.declare_dram_parameter("Wk", [E, E], f32, isOutput=False)
    Wv_d = nc.declare_dram_parameter("Wv", [E, E], f32, isOutput=False)
    Wo_d = nc.declare_dram_parameter("Wo", [E, E], f32, isOutput=False)
    bq_d = nc.declare_dram_parameter("bq", [E], f32, isOutput=False)
    bk_d = nc.declare_dram_parameter("bk", [E], f32, isOutput=False)
    bv_d = nc.declare_dram_parameter("bv", [E], f32, isOutput=False)
    bo_d = nc.declare_dram_parameter("bo", [E], f32, isOutput=False)
    out_d = nc.declare_dram_parameter("out", [S, E], f32, isOutput=True)

    with ExitStack() as ctx:
        tc = ctx.enter_context(tile.TileContext(nc))
        singles = ctx.enter_context(tc.tile_pool(name="singles", bufs=1))
        # 3 bufs: quarters 0/1/2 issue immediately on three separate queues
        # (sync/scalar/gpsimd); quarter 3 reuses quarter 0's buffer
        xld = ctx.enter_context(tc.tile_pool(name="xld", bufs=3))
        wqk = ctx.enter_context(tc.tile_pool(name="wqk", bufs=3))
        wbig = ctx.enter_context(tc.tile_pool(name="wbig", bufs=1))
        qkp = ctx.enter_context(tc.tile_pool(name="qkp", bufs=3))
        ep = ctx.enter_context(tc.tile_pool(name="ep", bufs=2))
        np_pool = ctx.enter_context(tc.tile_pool(name="norm", bufs=2))
        outp = ctx.enter_context(tc.tile_pool(name="outp", bufs=2))
        bcast = ctx.enter_context(tc.tile_pool(name="bcast", bufs=1))
        # PSUM: S (2 tiles x 2 banks) + mm (2x1) + att (2x1) = 8 banks
        psum = ctx.enter_context(tc.tile_pool(name="psum", bufs=2, space="PSUM"))
        psumS = ctx.enter_context(tc.tile_pool(name="psumS", bufs=2, space="PSUM"))

        # ---- constants / persistent buffers ----
        bq_sb = singles.tile([P, KT], f32)
        bk_sb = singles.tile([P, KT], f32)
        # small bias loads go via the gpsimd (SWDGE) queue so the sync queue
        # starts on the x tiles immediately
        nc.gpsimd.dma_start(bq_sb[:], bq_d[:].rearrange("(o p) -> p o", p=P))
        nc.gpsimd.dma_start(bk_sb[:], bk_d[:].rearrange("(o p) -> p o", p=P))

        def bcast_load(dst, src_ap):  # [E] -> [P, E] partition-step-0 DMA
            nc.gpsimd.dma_start(
                out=dst,
                in_=bass.AP(tensor=src_ap.tensor, offset=src_ap.offset,
                            ap=[[0, P]] + [list(a) for a in src_ap.ap]))
        bv_bc = bcast.tile([P, E], f32, tag="bbc")
        bcast_load(bv_bc[:], bv_d[:])

        xT = singles.tile([P, KT, S], mdt)          # x^T  [e_in, s]
        V_ext = singles.tile([P, ST, H, P], edt)    # [s, ktile, head, ...]
        concatT = singles.tile([P, NPAIR, S], mdt)  # attn^T by pair
        sel = singles.tile([2 * NPAIR, NPAIR, P], bf16)
        coll = [singles.tile([2 * NPAIR, QTILE], f32, name=f"coll{b}")
                for b in range(NQ)]

        ones_sb = singles.tile([P, P], bf16)

        def init_consts():
            # V_ext ones-halves + selector init; emitted AFTER the phase-1
            # x casts so the DVE queue prioritizes xT.  DVE partition bases
            # must be 32-aligned, so the selector's odd-partition one-blocks
            # are written via DMA from a ones tile instead of memsets.
            for h in range(H):
                c = 64 if h % 2 == 0 else 0
                nc.vector.memset(V_ext[:, :, h, c:c + D], 1.0)
            nc.vector.memset(ones_sb[:], 1.0)
            nc.vector.memset(sel[:], 0.0)
            for m in range(NPAIR):
                nc.sync.dma_start(sel[2 * m:2 * m + 1, m, 0:D],
                                  ones_sb[0:1, 0:D])
                nc.sync.dma_start(sel[2 * m + 1:2 * m + 2, m, D:P],
                                  ones_sb[0:1, 0:D])

        def wload(dst_r, src_slice_ap, stage_tag, dma=None, cast=None):
            """DMA f32 weights to a staging tile, cast into bf16 dst.

            Casts default to DVE; weights with slack (pairs 1-5, Wo) cast on
            gpsimd, which is idle during the attention phase.
            """
            stg = wqk.tile(list(dst_r.shape), f32, tag=stage_tag, name=stage_tag)
            (dma or nc.sync).dma_start(stg[:], src_slice_ap)
            (cast or nc.vector).tensor_copy(dst_r[:], stg[:])

        # ---- phase 1: transpose x on the PE ----
        # x arrives as four 0.75MB DMAs alternating sync/scalar queues; the
        # PE transposes per s-tile (XBAR DMA transpose into the strided xT
        # layout measured ~8x slower, and st-major xT slows the matmul rhs
        # streams).  Transpose PSUM tiles alternate between both psum pools
        # and the evacuation copies alternate DVE / ACT-Copy.
        ident = singles.tile([P, P], f32)
        make_identity(nc, ident)
        COPYF = mybir.ActivationFunctionType.Copy

        # HAM warm-up: the PE clock ramps to full rate only after ~4us of
        # continuous matmul work, and the prologue's first real matmuls
        # (transposes, pair-0 projection, first scores) otherwise run at
        # half clock.  Burn the DMA-wait window (~3-12us) on dummy
        # identity matmuls (fp32 = 4 cyc/col, so few instructions suffice).
        warm = psumS.tile([P, 2, 512], f32, tag="S", name="warm")
        for _ in range(14):
            nc.tensor.matmul(warm[:, 0, 0:P], ident[:], ident[:],
                             start=True, stop=True)

        def x_quarter(h, q, cast_eng=None):
            x_sb = xld.tile([P, 2, E], f32, tag="xs", name="x_sb")
            q.dma_start(x_sb[:], x_d[h * 256:(h + 1) * 256, :].rearrange(
                "(g p) e -> p g e", p=P))
            for g in range(2):
                st = h * 2 + g
                for e0, cnt in ((0, 4), (4, 2)):
                    # keep psumS free for the first scores tiles: transposes
                    # cycle mm + att tags instead
                    if st % 2 == 0:
                        pt = psum.tile([P, 512], f32, tag="mm", name="pt")
                    else:
                        pt = psum.tile([P, 512], f32, tag="att", name="ptA")
                    for j in range(cnt):
                        nc.tensor.transpose(
                            pt[:, j * P:(j + 1) * P],
                            x_sb[:, g, (e0 + j) * P:(e0 + j + 1) * P],
                            ident[:],
                        )
                    dst = xT[:, e0:e0 + cnt, st * P:(st + 1) * P]
                    src = pt[:, :cnt * P].rearrange("p (c s) -> p c s", c=cnt)
                    if e0 == 0:
                        nc.vector.tensor_copy(dst, src)
                    else:
                        nc.scalar.activation(dst, src, COPYF)

        x_quarter(0, nc.sync)
        x_quarter(1, nc.scalar)

        wq_t, wk_t, qt_t, kt_t = {}, {}, {}, {}
        Wq_re = Wq_d[:].rearrange("(ko p) m -> p ko m", p=P)
        Wk_re = Wk_d[:].rearrange("(ko p) m -> p ko m", p=P)

        def load_w(m, dma=None, cast=None):
            wq_t[m] = wqk.tile([P, KT, P], mdt, tag="wq", name="wq_m")
            wk_t[m] = wqk.tile([P, KT, P], mdt, tag="wk", name="wk_m")
            wload(wq_t[m], Wq_re[:, :, m * P:(m + 1) * P], "sq", dma, cast)
            wload(wk_t[m], Wk_re[:, :, m * P:(m + 1) * P], "sq", dma, cast)

        load_w(0, dma=nc.gpsimd)

        # big weights: one >=1MB DMA into a full staging tile, then 6 DVE
        # cast chunks.  SWDGE (gpsimd) reaches the same ~341GB/s.
        wstg_pool = ctx.enter_context(tc.tile_pool(name="wstg", bufs=1))
        Wv_sb = None

        def load_wbig(dst, src_re, q, cast=None):
            stg = wstg_pool.tile([P, KT, E], f32, tag="ws", name="wstage_big")
            q.dma_start(stg[:], src_re)
            for j in range(KT):
                (cast or nc.vector).tensor_copy(dst[:, j, :], stg[:, j, :])

        def load_wv():
            # casts on gpsimd: big DVE casts here were measured delaying the
            # kt-tile evacuation (and so the first scores) by ~4.5us
            nonlocal Wv_sb
            Wv_sb = wbig.tile([P, KT, E], mdt, tag="wbig", name="Wv_sb")
            load_wbig(Wv_sb, Wv_d[:].rearrange("(ko p) m -> p ko m", p=P),
                      nc.gpsimd, cast=nc.gpsimd)

        def v_chunk(nsi, st_range):
            noff, nsz = ESLICES[nsi]
            npr = nsz // P
            for st in st_range:
                pv = psum.tile([P, 512], f32, tag="mm", name="pv")
                for k in range(KT):
                    nc.tensor.matmul(
                        pv[:, :nsz],
                        xT[:, k, st * P:(st + 1) * P],
                        Wv_sb[:, k, noff:noff + nsz],
                        start=(k == 0), stop=(k == KT - 1),
                    )
                # batched bias-add evacuation: one op per head parity
                # (even heads -> cols 0:64, odd heads -> cols 64:128)
                dst = V_ext[:, st, 8 * nsi:8 * nsi + 2 * npr, :].rearrange(
                    "p (pr two) c -> p pr two c", two=2)
                src = pv[:, :nsz].rearrange("p (pr two c) -> p pr two c",
                                            two=2, c=D)
                bias = bv_bc[:, noff:noff + nsz].rearrange(
                    "p (pr two c) -> p pr two c", two=2, c=D)
                nc.vector.tensor_add(dst[:, :, 0, 0:D], src[:, :, 0, :],
                                     bias[:, :, 0, :])
                nc.vector.tensor_add(dst[:, :, 1, D:P], src[:, :, 1, :],
                                     bias[:, :, 1, :])

        # ---- attention: 12 software-pipelined units ----
        def proj_half(m, which, q2):
            """6 matmuls: half of QT_m (or KT_m)."""
            for f in micros_proj(m, which, q2):
                f()

        def micros_proj(m, which, q2):
            """Two ~3-matmul micro-ops (must be emitted consecutively)."""
            state = {}
            qsl = slice(q2 * QTILE, (q2 + 1) * QTILE)

            def a():
                w = wq_t[m] if which == "q" else wk_t[m]
                tiles = qt_t if which == "q" else kt_t
                if m not in tiles:
                    tiles[m] = qkp.tile([P, S], mdt, tag=which + "t",
                                        name=which + "t")
                state["pq"] = psum.tile([P, 512], f32, tag="mm", name="pq")
                for k in range(3):
                    nc.tensor.matmul(state["pq"][:], w[:, k, :], xT[:, k, qsl],
                                     start=(k == 0), stop=False)

            def b():
                w = wq_t[m] if which == "q" else wk_t[m]
                bias = bq_sb if which == "q" else bk_sb
                t = (qt_t if which == "q" else kt_t)[m]
                pq = state["pq"]
                for k in range(3, KT):
                    nc.tensor.matmul(pq[:], w[:, k, :], xT[:, k, qsl],
                                     start=False, stop=(k == KT - 1))
                nc.vector.tensor_scalar_add(t[:, qsl], pq[:], bias[:, m:m + 1])

            return [a, b]

        def micro_v(nsi, st):
            return lambda: v_chunk(nsi, range(st, st + 1))

        # pairs 0-2 interleave q-halves; pairs 3-5 run q2=0 before q2=1 so
        # the q2=0 normalization + first output half overlap the last exps
        units = [(0, 0), (0, 1), (1, 0), (1, 1), (2, 0), (2, 1),
                 (3, 0), (4, 0), (5, 0), (3, 1), (4, 1), (5, 1)]
        e_tiles = {}
        att_ps = {}

        def scores_c(i, c):
            m, q2 = units[i]
            qsl = slice(q2 * QTILE, (q2 + 1) * QTILE)
            qt_m, kt_m = qt_t[m], kt_t[m]
            if c == 0:
                e_tiles[i] = (
                    ep.tile([P, ST, QTILE], edt, tag="eA", name="e_a"),
                    ep.tile([P, ST, QTILE], edt, tag="eB", name="e_b"),
                )
            e_a, e_b = e_tiles[i]
            s_a = psumS.tile([P, 2, 512], f32, tag="S", name="s_a")
            s_b = psumS.tile([P, 2, 512], f32, tag="S", name="s_b")
            for kk in range(2):
                ktile = c * 2 + kk
                ksl = slice(ktile * P, (ktile + 1) * P)
                nc.tensor.matmul(s_a[:, kk, :], kt_m[0:D, ksl],
                                 qt_m[0:D, qsl], start=True, stop=True)
                nc.tensor.matmul(s_b[:, kk, :], kt_m[D:P, ksl],
                                 qt_m[D:P, qsl], start=True, stop=True)
            nc.scalar.activation(e_a[:, c * 2:c * 2 + 2, :], s_a[:], EXP, scale=0.125)
            nc.scalar.activation(e_b[:, c * 2:c * 2 + 2, :], s_b[:], EXP, scale=0.125)

        def attnv_chunk(i, kts):
            m, q2 = units[i]
            e_a, e_b = e_tiles[i]
            if kts[0] == 0:
                att_ps[i] = (psum.tile([P, 512], f32, tag="att", name="p_a"),
                             psum.tile([P, 512], f32, tag="att", name="p_b"))
            p_a, p_b = att_ps[i]
            for ktile in kts:
                nc.tensor.matmul(p_a[:], V_ext[:, ktile, 2 * m, :],
                                 e_a[:, ktile, :],
                                 start=(ktile == 0), stop=(ktile == ST - 1))
            for ktile in kts:
                nc.tensor.matmul(p_b[:], V_ext[:, ktile, 2 * m + 1, :],
                                 e_b[:, ktile, :],
                                 start=(ktile == 0), stop=(ktile == ST - 1))

        def attnv_finish(i):
            m, q2 = units[i]
            qsl = slice(q2 * QTILE, (q2 + 1) * QTILE)
            p_a, p_b = att_ps[i]
            # evacuate attn halves straight into concatT (unnormalized);
            # partition-aligned DVE copies (PSUM-read alignment rule)
            nc.vector.tensor_copy(concatT[0:D, m, qsl], p_a[0:D, :])
            nc.vector.tensor_copy(concatT[D:P, m, qsl], p_b[D:P, :])
            # one sums row per head (rows 64+ of p_a / rows 0:64 of p_b are
            # 64 identical sums rows each).  DMA cannot read PSUM, so hop
            # through partition-aligned DVE copies into an SBUF scratch row.
            srow = np_pool.tile([P, QTILE], f32, tag="srow", name="srow")
            nc.vector.tensor_copy(srow[D:D + 1, :], p_a[D:D + 1, :])
            nc.vector.tensor_copy(srow[0:1, :], p_b[0:1, :])
            nc.sync.dma_start(coll[q2][2 * m:2 * m + 1, :], srow[D:D + 1, :])
            nc.sync.dma_start(coll[q2][2 * m + 1:2 * m + 2, :], srow[0:1, :])

        def micros_out(st):
            """Three micro-ops per output s-tile (emit in order)."""
            state = {}
            ssl = slice(st * P, (st + 1) * P)

            def a():
                state["o"] = outp.tile([P, E], f32, tag="o", name="o_sb")
                state["po"] = psum.tile([P, 512], f32, tag="mm", name="po")
                for k in range(3):
                    nc.tensor.matmul(state["po"][:], concatT[:, k, ssl],
                                     Wo_sb[:, k, 0:512],
                                     start=(k == 0), stop=False)

            def b():
                po = state["po"]
                for k in range(3, KT):
                    nc.tensor.matmul(po[:], concatT[:, k, ssl],
                                     Wo_sb[:, k, 0:512],
                                     start=False, stop=(k == KT - 1))
                nc.vector.tensor_add(state["o"][:, 0:512], po[:],
                                     bo_bc[:, 0:512])

            def c_():
                po = psum.tile([P, 512], f32, tag="mm", name="po2")
                for k in range(KT):
                    nc.tensor.matmul(po[:, :256], concatT[:, k, ssl],
                                     Wo_sb[:, k, 512:768],
                                     start=(k == 0), stop=(k == KT - 1))
                nc.vector.tensor_add(state["o"][:, 512:768], po[:, :256],
                                     bo_bc[:, 512:768])
                (nc.sync if st % 2 == 0 else nc.scalar).dma_start(
                    out_d[ssl, :], state["o"][:])

            return [a, b, c_]

        def micros_norm(q2):
            """[reciprocal+cast (DVE only), bcast+mul pairs 0-2, pairs 3-5]."""
            state = {}
            qsl = slice(q2 * QTILE, (q2 + 1) * QTILE)

            def r():
                c_t = coll[q2]
                nc.vector.reciprocal(c_t[:], c_t[:])
                state["rec"] = np_pool.tile([2 * NPAIR, QTILE], bf16,
                                            tag="recr", name="rec_r")
                nc.vector.tensor_copy(state["rec"][:], c_t[:])

            def mk(ms):
                def f():
                    for m in ms:
                        pb = psum.tile([P, 512], f32, tag="mm", name="pb")
                        nc.tensor.matmul(pb[:], sel[:, m, :], state["rec"][:],
                                         start=True, stop=True)
                        nc.vector.tensor_mul(concatT[:, m, qsl],
                                             concatT[:, m, qsl], pb[:])
                return f

            return [r, mk([0, 1, 2]), mk([3, 4, 5])]

        # prologue: pair-0 q2=0 halves right after xT sts 0-3 land (they only
        # need those four s-tiles + wq0/wk0), THEN the second x half's
        # transposes — so scores(0)/exp(0) start ~25us earlier.
        proj_half(0, "q", 0)
        proj_half(0, "k", 0)
        x_quarter(2, nc.gpsimd)
        x_quarter(3, nc.scalar)
        load_wv()
        init_consts()
        load_w(1, cast=nc.gpsimd)
        # fill the prologue's DMA-wait PE gaps with the first V-pass tiles
        v_chunk(0, range(0, 2))

        Wo_sb = None
        bo_bc = None

        def stage_wo():
            nonlocal Wo_sb, bo_bc
            bo_bc = bcast.tile([P, E], f32, tag="bbc", name="bo_bc")
            bcast_load(bo_bc[:], bo_d[:])
            # ACT is the attention-phase bottleneck: keep Wo's DMA issue off
            # the scalar queue (gpsimd SWDGE is idle here)
            Wo_sb = wbig.tile([P, KT, E], mdt, tag="wbig", name="Wo_sb")
            load_wbig(Wo_sb, Wo_d[:].rearrange("(ko p) m -> p ko m", p=P),
                      nc.gpsimd, cast=nc.gpsimd)

        # Micro-op fillers (~3 matmuls each): one is emitted after every
        # scores/attnV quarter so the PE always has independent work while
        # ACT drains the score PSUM tiles; leftovers emit after c3.
        # Deadlines: pair m's proj micros land before their consuming
        # scores; V chunks stay one attnV-chunk ahead; norm0/out st0-2
        # overlap the last two exps.
        n0 = micros_norm(0)
        fillers = {
            0: micros_proj(0, "k", 1) + micros_proj(0, "q", 1)
               + [micro_v(0, 2), micro_v(0, 3), micro_v(0, 4), micro_v(0, 5)],
            1: [micro_v(0, 6), micro_v(0, 7)]
               + micros_proj(1, "q", 0) + micros_proj(1, "k", 0),
            2: micros_proj(1, "k", 1) + micros_proj(1, "q", 1),
            3: micros_proj(2, "q", 0) + micros_proj(2, "k", 0)
               + [stage_wo, micro_v(1, 0), micro_v(1, 1)],
            4: micros_proj(2, "k", 1) + micros_proj(2, "q", 1)
               + [micro_v(1, 2), micro_v(1, 3)],
            5: micros_proj(3, "q", 0) + micros_proj(3, "k", 0)
               + [micro_v(1, 4), micro_v(1, 5)],
            6: micros_proj(3, "k", 1) + micros_proj(4, "q", 0)
               + micros_proj(4, "k", 0) + [micro_v(1, 6)],
            7: micros_proj(4, "k", 1) + micros_proj(5, "q", 0)
               + micros_proj(5, "k", 0) + [micro_v(1, 7)],
            8: micros_proj(5, "k", 1) + micros_proj(3, "q", 1),
            9: micros_proj(4, "q", 1) + micros_proj(5, "q", 1),
            10: [n0[1], n0[2]] + micros_out(0),
            11: micros_out(1) + micros_out(2) + micros_out(3),
        }
        loadw_at = {1: 2, 3: 3, 4: 4, 5: 5}
        post = {9: [n0[0]]}

        for i in range(len(units)):
            if i in loadw_at:
                load_w(loadw_at[i], cast=nc.gpsimd)
            chunks = list(fillers.get(i, []))
            prev = i - 1
            for c in range(4):
                scores_c(i, c)
                if prev >= 0:
                    attnv_chunk(prev, (2 * c, 2 * c + 1))
                if c < len(chunks):
                    chunks[c]()
            for ch in chunks[4:]:
                ch()
            if prev >= 0:
                attnv_finish(prev)
            for p_fn in post.get(i, []):
                p_fn()
        # tail: last attnV, q2=1 normalization, second output half
        for c in range(4):
            attnv_chunk(11, (2 * c, 2 * c + 1))
        attnv_finish(11)
        for f in micros_norm(1):
            f()
        for st in range(4, 8):
            for f in micros_out(st):
                f()

    _split_excess_waits(nc)
    return nc


def run_spmd(inputs, Wq, bq, Wk, bk, Wv, bv, Wo, bo,
             mm_dtype="bf16", e_dtype="bf16", crossbase=False, trace=False):
    key = (mm_dtype, e_dtype)
    if key not in _NC_CACHE:
        _NC_CACHE[key] = build(mm_dtype, e_dtype)
    nc = _NC_CACHE[key]
    x = np.asarray(inputs, dtype=np.float32)
    common = {
        "Wq": np.asarray(Wq, np.float32), "Wk": np.asarray(Wk, np.float32),
        "Wv": np.asarray(Wv, np.float32), "Wo": np.asarray(Wo, np.float32),
        "bq": np.asarray(bq, np.float32), "bk": np.asarray(bk, np.float32),
        "bv": np.asarray(bv, np.float32), "bo": np.asarray(bo, np.float32),
    }
    in_maps = [dict(common, x=np.ascontiguousarray(x[b])) for b in range(x.shape[0])]
    res = run_bass_kernel_spmd(nc, in_maps, core_ids=list(range(len(in_maps))),
                               trace=trace)
    out = np.stack([res.results[b]["out"] for b in range(len(in_maps))], axis=0)
    return out, res


def kernel(inputs, Wq, bq, Wk, bk, Wv, bv, Wo, bo):
    out, _ = run_spmd(inputs, Wq, bq, Wk, bk, Wv, bv, Wo, bo)
    return out


# revision 83
# speedup vs baseline: 1.0777x; 1.0777x over previous
"""Multi-head self-attention (B=8, S=1024, E=768, H=12, D=64) on 8 NeuronCores.

Sharding: data-parallel over batch — one batch element per core, weights
replicated, no collectives.  Measured ~216us HW exec (vs 263us f32r
baseline), rel-rms err 5.8e-3 vs the fp32 reference.

The kernel is jointly limited by the PE (~179us of matmul stream; bf16
runs 2 cols/cycle, fp8 is numerically infeasible here) and ACT (~107us of
exp; (N+352)/1.2ns per activation).  Everything else is scheduled around
keeping both dense:

  1. x arrives as four 0.75MB DMAs alternating the sync/scalar issue
     queues (each hwdge engine owns one ~80-130GB/s queue; >=1MB batches
     are fastest).  The PE transposes x per s-tile (XBAR DMA transpose
     into the strided xT layout measured ~8x slower); evacuation copies
     alternate DVE / ACT-Copy so no single engine paces the pipeline.
  2. V_ext[s, ktile, head, 128] = [V_h+bv | ones] (even head) or
     [ones | V_h+bv] (odd head); bv folded exactly (softmax rows sum to
     1).  The ones columns make each attnV matmul also produce the
     softmax denominators for free (M doesn't affect PE time).
  3. Attention runs as 12 software-pipelined units (pair m, q-half q2),
     ordered (0,0),(0,1),...,(2,1),(3,0),(4,0),(5,0),(3,1),(4,1),(5,1) so
     the q2=0 normalization + first output tiles overlap the last exps.
     PSUM holds only 2 of the 8 score tiles per unit, so PE emission
     interleaves at quarter-unit granularity: scores(i,c) / attnV(i-1)
     2-ktile chunks / ~3-matmul filler micro-ops (projection chain
     halves, V-pass tiles, output tiles, normalization broadcasts).
  4. Softmax normalization is deferred and batched per q-half: one sums
     row per head is staged (aligned DVE row copy, PSUM->SBUF) and DMA'd
     into a [12, 512] collect tile; ONE DVE reciprocal per batch
     (reciprocal cost is per-free-element, so batching 6 units is ~6x
     cheaper; reciprocal_approx_fast hits "ISA wrong length" in this
     walrus build), then a K=12 selector matmul broadcasts the
     reciprocals across partitions and a DVE multiply normalizes concatT
     in place.
  5. out = concatT.T @ Wo + bo; output stores stripe across both hwdge
     queues.  Weight casts f32->bf16 run on DVE when on the critical path
     and on the otherwise-idle gpsimd during the attention phase.

Numerics: bf16 operands everywhere, fp32 PSUM accumulation; exp folds the
1/sqrt(D)=1/8 scale (scores ~N(0,1), no max subtraction needed).

Hardware workarounds: this walrus build caps sync waits per instruction
(_split_excess_waits hoists the surplus onto standalone EVSEM ops); DVE
ops need 32-aligned partition bases and must not read PSUM at a partition
base different from their output's; custom DVE ops and
--enable-ldw-opt=true are broken in this build and avoided.
"""
import sys
sys.path.insert(0, "/opt/trn_rl_repo")
from contextlib import ExitStack

import numpy as np

import concourse.bass as bass
import concourse.tile as tile
from concourse import mybir
from concourse.bass_utils import run_bass_kernel_spmd
from concourse.masks import make_identity
from concourse.vector_clock import ScopedClock




def _split_drain_and_barrier(self, tick_clock, wait_clock):
    """TileContext tail with the final drain's waits split one-per-instruction."""
    drain_inst = self.nc.sync.drain()
    wait_clock.add_sem_waits(
        drain_inst.ins, ScopedClock({None: tick_clock.global_clock})
    )
    si = drain_inst.ins.sync_info
    waits = list(si.on_wait) if si is not None and si.on_wait else []
    if len(waits) > 1:
        si.on_wait = []
        by_num = {h.num: h for h in self.sems.allocated().values()}
        for w in waits:
            self.nc.sync.wait_ge(by_num[w.id], w.wait_value)
    self.nc.all_engine_barrier()
    popped = self.nc._tile_sem_poison_stack.pop()
    assert popped is self._sem_poison
    self.nc.clear_and_free_semaphores(list(self.sems.allocated().values()))
    self.nc.all_engine_barrier()


tile.TileContext._drain_and_barrier = _split_drain_and_barrier


def _split_excess_waits(nc):
    """Hoist excess per-instruction sync waits into standalone EVSEM waits."""
    counter = 0
    for f in nc.m.functions:
        for bb in f.blocks:
            insts = bb.instructions
            out = []
            for inst in insts:
                si = inst.sync_info
                cap = 2 if isinstance(inst, mybir.InstEventSemaphore) else 1
                if si is not None and si.on_wait and len(si.on_wait) > cap:
                    waits = list(si.on_wait)
                    for w in waits[cap:]:
                        counter += 1
                        ev = mybir.InstEventSemaphore(name=f"I-wsplit-{counter}")
                        ev.engine = inst.engine
                        ev.sync_info = mybir.SyncInfo(on_wait=[w], on_update=[])
                        out.append(ev)
                    si.on_wait = waits[:cap]
                out.append(inst)
            if len(out) != len(insts):
                insts[:] = out
    return counter

P = 128
S = 1024
E = 768
H = 12
D = 64
KT = E // P        # 6 e-tiles
ST = S // P        # 8 s-tiles
NPAIR = H // 2     # 6 head pairs
QTILE = 512
NQ = S // QTILE    # 2 q-tiles
ESLICES = [(0, 512), (512, 256)]

f32 = mybir.dt.float32
f32r = mybir.dt.float32r
bf16 = mybir.dt.bfloat16
EXP = mybir.ActivationFunctionType.Exp

_NC_CACHE = {}


def build(mm_dtype="bf16", e_dtype="bf16"):
    assert mm_dtype == "bf16" and e_dtype == "bf16"
    mdt = bf16
    edt = bf16
    nc = bass.Bass()
    x_d = nc.declare_dram_parameter("x", [S, E], f32, isOutput=False)
    Wq_d = nc.declare_dram_parameter("Wq", [E, E], f32, isOutput=False)
    Wk_d = nc.declare_dram_parameter("Wk", [E, E], f32, isOutput=False)
    Wv_d = nc.declare_dram_parameter("Wv", [E, E], f32, isOutput=False)
    Wo_d = nc.declare_dram_parameter("Wo", [E, E], f32, isOutput=False)
    bq_d = nc.declare_dram_parameter("bq", [E], f32, isOutput=False)
    bk_d = nc.declare_dram_parameter("bk", [E], f32, isOutput=False)
    bv_d = nc.declare_dram_parameter("bv", [E], f32, isOutput=False)
    bo_d = nc.declare_dram_parameter("bo", [E], f32, isOutput=False)
    out_d = nc.declare_dram_parameter("out", [S, E], f32, isOutput=True)

    with ExitStack() as ctx:
        tc = ctx.enter_context(tile.TileContext(nc))
        singles = ctx.enter_context(tc.tile_pool(name="singles", bufs=1))
        xld = ctx.enter_context(tc.tile_pool(name="xld", bufs=2))
        wqk = ctx.enter_context(tc.tile_pool(name="wqk", bufs=3))
        wbig = ctx.enter_context(tc.tile_pool(name="wbig", bufs=1))
        qkp = ctx.enter_context(tc.tile_pool(name="qkp", bufs=3))
        ep = ctx.enter_context(tc.tile_pool(name="ep", bufs=2))
        np_pool = ctx.enter_context(tc.tile_pool(name="norm", bufs=2))
        outp = ctx.enter_context(tc.tile_pool(name="outp", bufs=2))
        bcast = ctx.enter_context(tc.tile_pool(name="bcast", bufs=1))
        # PSUM: S (2 tiles x 2 banks) + mm (2x1) + att (2x1) = 8 banks
        psum = ctx.enter_context(tc.tile_pool(name="psum", bufs=2, space="PSUM"))
        psumS = ctx.enter_context(tc.tile_pool(name="psumS", bufs=2, space="PSUM"))

        # ---- constants / persistent buffers ----
        bq_sb = singles.tile([P, KT], f32)
        bk_sb = singles.tile([P, KT], f32)
        # small bias loads go via the gpsimd (SWDGE) queue so the sync queue
        # starts on the x tiles immediately
        nc.gpsimd.dma_start(bq_sb[:], bq_d[:].rearrange("(o p) -> p o", p=P))
        nc.gpsimd.dma_start(bk_sb[:], bk_d[:].rearrange("(o p) -> p o", p=P))

        def bcast_load(dst, src_ap):  # [E] -> [P, E] partition-step-0 DMA
            nc.gpsimd.dma_start(
                out=dst,
                in_=bass.AP(tensor=src_ap.tensor, offset=src_ap.offset,
                            ap=[[0, P]] + [list(a) for a in src_ap.ap]))
        bv_bc = bcast.tile([P, E], f32, tag="bbc")
        bcast_load(bv_bc[:], bv_d[:])

        xT = singles.tile([P, KT, S], mdt)          # x^T  [e_in, s]
        V_ext = singles.tile([P, ST, H, P], edt)    # [s, ktile, head, ...]
        concatT = singles.tile([P, NPAIR, S], mdt)  # attn^T by pair
        sel = singles.tile([2 * NPAIR, NPAIR, P], bf16)
        coll = [singles.tile([2 * NPAIR, QTILE], f32, name=f"coll{b}")
                for b in range(NQ)]

        ones_sb = singles.tile([P, P], bf16)

        def init_consts():
            # V_ext ones-halves + selector init; emitted AFTER the phase-1
            # x casts so the DVE queue prioritizes xT.  DVE partition bases
            # must be 32-aligned, so the selector's odd-partition one-blocks
            # are written via DMA from a ones tile instead of memsets.
            for h in range(H):
                c = 64 if h % 2 == 0 else 0
                nc.vector.memset(V_ext[:, :, h, c:c + D], 1.0)
            nc.vector.memset(ones_sb[:], 1.0)
            nc.vector.memset(sel[:], 0.0)
            for m in range(NPAIR):
                nc.sync.dma_start(sel[2 * m:2 * m + 1, m, 0:D],
                                  ones_sb[0:1, 0:D])
                nc.sync.dma_start(sel[2 * m + 1:2 * m + 2, m, D:P],
                                  ones_sb[0:1, 0:D])

        def wload(dst_r, src_slice_ap, stage_tag, dma=None, cast=None):
            """DMA f32 weights to a staging tile, cast into bf16 dst.

            Casts default to DVE; weights with slack (pairs 1-5, Wo) cast on
            gpsimd, which is idle during the attention phase.
            """
            stg = wqk.tile(list(dst_r.shape), f32, tag=stage_tag, name=stage_tag)
            (dma or nc.sync).dma_start(stg[:], src_slice_ap)
            (cast or nc.vector).tensor_copy(dst_r[:], stg[:])

        # ---- phase 1: transpose x on the PE ----
        # x arrives as four 0.75MB DMAs alternating sync/scalar queues; the
        # PE transposes per s-tile (XBAR DMA transpose into the strided xT
        # layout measured ~8x slower, and st-major xT slows the matmul rhs
        # streams).  Transpose PSUM tiles alternate between both psum pools
        # and the evacuation copies alternate DVE / ACT-Copy.
        ident = singles.tile([P, P], f32)
        make_identity(nc, ident)
        COPYF = mybir.ActivationFunctionType.Copy

        # HAM warm-up: the PE clock ramps to full rate only after ~4us of
        # continuous matmul work, and the prologue's first real matmuls
        # (transposes, pair-0 projection, first scores) otherwise run at
        # half clock.  Burn the DMA-wait window (~3-12us) on dummy
        # identity matmuls (fp32 = 4 cyc/col, so few instructions suffice).
        warm = psumS.tile([P, 2, 512], f32, tag="S", name="warm")
        for _ in range(14):
            nc.tensor.matmul(warm[:, 0, 0:P], ident[:], ident[:],
                             start=True, stop=True)

        def x_quarter(h, q, cast_eng=None):
            x_sb = xld.tile([P, 2, E], f32, tag="xs", name="x_sb")
            q.dma_start(x_sb[:], x_d[h * 256:(h + 1) * 256, :].rearrange(
                "(g p) e -> p g e", p=P))
            for g in range(2):
                st = h * 2 + g
                for e0, cnt in ((0, 4), (4, 2)):
                    # keep psumS free for the first scores tiles: transposes
                    # cycle mm + att tags instead
                    if st % 2 == 0:
                        pt = psum.tile([P, 512], f32, tag="mm", name="pt")
                    else:
                        pt = psum.tile([P, 512], f32, tag="att", name="ptA")
                    for j in range(cnt):
                        nc.tensor.transpose(
                            pt[:, j * P:(j + 1) * P],
                            x_sb[:, g, (e0 + j) * P:(e0 + j + 1) * P],
                            ident[:],
                        )
                    dst = xT[:, e0:e0 + cnt, st * P:(st + 1) * P]
                    src = pt[:, :cnt * P].rearrange("p (c s) -> p c s", c=cnt)
                    if e0 == 0:
                        nc.vector.tensor_copy(dst, src)
                    else:
                        nc.scalar.activation(dst, src, COPYF)

        x_quarter(0, nc.sync)
        x_quarter(1, nc.scalar)

        wq_t, wk_t, qt_t, kt_t = {}, {}, {}, {}
        Wq_re = Wq_d[:].rearrange("(ko p) m -> p ko m", p=P)
        Wk_re = Wk_d[:].rearrange("(ko p) m -> p ko m", p=P)

        def load_w(m, dma=None, cast=None):
            wq_t[m] = wqk.tile([P, KT, P], mdt, tag="wq", name="wq_m")
            wk_t[m] = wqk.tile([P, KT, P], mdt, tag="wk", name="wk_m")
            wload(wq_t[m], Wq_re[:, :, m * P:(m + 1) * P], "sq", dma, cast)
            wload(wk_t[m], Wk_re[:, :, m * P:(m + 1) * P], "sq", dma, cast)

        load_w(0, dma=nc.gpsimd)

        # big weights: one >=1MB DMA into a full staging tile, then 6 DVE
        # cast chunks.  SWDGE (gpsimd) reaches the same ~341GB/s.
        wstg_pool = ctx.enter_context(tc.tile_pool(name="wstg", bufs=1))
        Wv_sb = None

        def load_wbig(dst, src_re, q, cast=None):
            stg = wstg_pool.tile([P, KT, E], f32, tag="ws", name="wstage_big")
            q.dma_start(stg[:], src_re)
            for j in range(KT):
                (cast or nc.vector).tensor_copy(dst[:, j, :], stg[:, j, :])

        def load_wv():
            nonlocal Wv_sb
            Wv_sb = wbig.tile([P, KT, E], mdt, tag="wbig", name="Wv_sb")
            load_wbig(Wv_sb, Wv_d[:].rearrange("(ko p) m -> p ko m", p=P),
                      nc.gpsimd)

        def v_chunk(nsi, st_range):
            noff, nsz = ESLICES[nsi]
            npr = nsz // P
            for st in st_range:
                pv = psum.tile([P, 512], f32, tag="mm", name="pv")
                for k in range(KT):
                    nc.tensor.matmul(
                        pv[:, :nsz],
                        xT[:, k, st * P:(st + 1) * P],
                        Wv_sb[:, k, noff:noff + nsz],
                        start=(k == 0), stop=(k == KT - 1),
                    )
                # batched bias-add evacuation: one op per head parity
                # (even heads -> cols 0:64, odd heads -> cols 64:128)
                dst = V_ext[:, st, 8 * nsi:8 * nsi + 2 * npr, :].rearrange(
                    "p (pr two) c -> p pr two c", two=2)
                src = pv[:, :nsz].rearrange("p (pr two c) -> p pr two c",
                                            two=2, c=D)
                bias = bv_bc[:, noff:noff + nsz].rearrange(
                    "p (pr two c) -> p pr two c", two=2, c=D)
                nc.vector.tensor_add(dst[:, :, 0, 0:D], src[:, :, 0, :],
                                     bias[:, :, 0, :])
                nc.vector.tensor_add(dst[:, :, 1, D:P], src[:, :, 1, :],
                                     bias[:, :, 1, :])

        # ---- attention: 12 software-pipelined units ----
        def proj_half(m, which, q2):
            """6 matmuls: half of QT_m (or KT_m)."""
            for f in micros_proj(m, which, q2):
                f()

        def micros_proj(m, which, q2):
            """Two ~3-matmul micro-ops (must be emitted consecutively)."""
            state = {}
            qsl = slice(q2 * QTILE, (q2 + 1) * QTILE)

            def a():
                w = wq_t[m] if which == "q" else wk_t[m]
                tiles = qt_t if which == "q" else kt_t
                if m not in tiles:
                    tiles[m] = qkp.tile([P, S], mdt, tag=which + "t",
                                        name=which + "t")
                state["pq"] = psum.tile([P, 512], f32, tag="mm", name="pq")
                for k in range(3):
                    nc.tensor.matmul(state["pq"][:], w[:, k, :], xT[:, k, qsl],
                                     start=(k == 0), stop=False)

            def b():
                w = wq_t[m] if which == "q" else wk_t[m]
                bias = bq_sb if which == "q" else bk_sb
                t = (qt_t if which == "q" else kt_t)[m]
                pq = state["pq"]
                for k in range(3, KT):
                    nc.tensor.matmul(pq[:], w[:, k, :], xT[:, k, qsl],
                                     start=False, stop=(k == KT - 1))
                nc.vector.tensor_scalar_add(t[:, qsl], pq[:], bias[:, m:m + 1])

            return [a, b]

        def micro_v(nsi, st):
            return lambda: v_chunk(nsi, range(st, st + 1))

        # pairs 0-2 interleave q-halves; pairs 3-5 run q2=0 before q2=1 so
        # the q2=0 normalization + first output half overlap the last exps
        units = [(0, 0), (0, 1), (1, 0), (1, 1), (2, 0), (2, 1),
                 (3, 0), (4, 0), (5, 0), (3, 1), (4, 1), (5, 1)]
        e_tiles = {}
        att_ps = {}

        def scores_c(i, c):
            m, q2 = units[i]
            qsl = slice(q2 * QTILE, (q2 + 1) * QTILE)
            qt_m, kt_m = qt_t[m], kt_t[m]
            if c == 0:
                e_tiles[i] = (
                    ep.tile([P, ST, QTILE], edt, tag="eA", name="e_a"),
                    ep.tile([P, ST, QTILE], edt, tag="eB", name="e_b"),
                )
            e_a, e_b = e_tiles[i]
            s_a = psumS.tile([P, 2, 512], f32, tag="S", name="s_a")
            s_b = psumS.tile([P, 2, 512], f32, tag="S", name="s_b")
            for kk in range(2):
                ktile = c * 2 + kk
                ksl = slice(ktile * P, (ktile + 1) * P)
                nc.tensor.matmul(s_a[:, kk, :], kt_m[0:D, ksl],
                                 qt_m[0:D, qsl], start=True, stop=True)
                nc.tensor.matmul(s_b[:, kk, :], kt_m[D:P, ksl],
                                 qt_m[D:P, qsl], start=True, stop=True)
            nc.scalar.activation(e_a[:, c * 2:c * 2 + 2, :], s_a[:], EXP, scale=0.125)
            nc.scalar.activation(e_b[:, c * 2:c * 2 + 2, :], s_b[:], EXP, scale=0.125)

        def attnv_chunk(i, kts):
            m, q2 = units[i]
            e_a, e_b = e_tiles[i]
            if kts[0] == 0:
                att_ps[i] = (psum.tile([P, 512], f32, tag="att", name="p_a"),
                             psum.tile([P, 512], f32, tag="att", name="p_b"))
            p_a, p_b = att_ps[i]
            for ktile in kts:
                nc.tensor.matmul(p_a[:], V_ext[:, ktile, 2 * m, :],
                                 e_a[:, ktile, :],
                                 start=(ktile == 0), stop=(ktile == ST - 1))
            for ktile in kts:
                nc.tensor.matmul(p_b[:], V_ext[:, ktile, 2 * m + 1, :],
                                 e_b[:, ktile, :],
                                 start=(ktile == 0), stop=(ktile == ST - 1))

        def attnv_finish(i):
            m, q2 = units[i]
            qsl = slice(q2 * QTILE, (q2 + 1) * QTILE)
            p_a, p_b = att_ps[i]
            # evacuate attn halves straight into concatT (unnormalized);
            # partition-aligned DVE copies (PSUM-read alignment rule)
            nc.vector.tensor_copy(concatT[0:D, m, qsl], p_a[0:D, :])
            nc.vector.tensor_copy(concatT[D:P, m, qsl], p_b[D:P, :])
            # one sums row per head (rows 64+ of p_a / rows 0:64 of p_b are
            # 64 identical sums rows each).  DMA cannot read PSUM, so hop
            # through partition-aligned DVE copies into an SBUF scratch row.
            srow = np_pool.tile([P, QTILE], f32, tag="srow", name="srow")
            nc.vector.tensor_copy(srow[D:D + 1, :], p_a[D:D + 1, :])
            nc.vector.tensor_copy(srow[0:1, :], p_b[0:1, :])
            nc.sync.dma_start(coll[q2][2 * m:2 * m + 1, :], srow[D:D + 1, :])
            nc.sync.dma_start(coll[q2][2 * m + 1:2 * m + 2, :], srow[0:1, :])

        def micros_out(st):
            """Three micro-ops per output s-tile (emit in order)."""
            state = {}
            ssl = slice(st * P, (st + 1) * P)

            def a():
                state["o"] = outp.tile([P, E], f32, tag="o", name="o_sb")
                state["po"] = psum.tile([P, 512], f32, tag="mm", name="po")
                for k in range(3):
                    nc.tensor.matmul(state["po"][:], concatT[:, k, ssl],
                                     Wo_sb[:, k, 0:512],
                                     start=(k == 0), stop=False)

            def b():
                po = state["po"]
                for k in range(3, KT):
                    nc.tensor.matmul(po[:], concatT[:, k, ssl],
                                     Wo_sb[:, k, 0:512],
                                     start=False, stop=(k == KT - 1))
                nc.vector.tensor_add(state["o"][:, 0:512], po[:],
                                     bo_bc[:, 0:512])

            def c_():
                po = psum.tile([P, 512], f32, tag="mm", name="po2")
                for k in range(KT):
                    nc.tensor.matmul(po[:, :256], concatT[:, k, ssl],
                                     Wo_sb[:, k, 512:768],
                                     start=(k == 0), stop=(k == KT - 1))
                nc.vector.tensor_add(state["o"][:, 512:768], po[:, :256],
                                     bo_bc[:, 512:768])
                (nc.sync if st % 2 == 0 else nc.scalar).dma_start(
                    out_d[ssl, :], state["o"][:])

            return [a, b, c_]

        def micros_norm(q2):
            """[reciprocal+cast (DVE only), bcast+mul pairs 0-2, pairs 3-5]."""
            state = {}
            qsl = slice(q2 * QTILE, (q2 + 1) * QTILE)

            def r():
                c_t = coll[q2]
                nc.vector.reciprocal(c_t[:], c_t[:])
                state["rec"] = np_pool.tile([2 * NPAIR, QTILE], bf16,
                                            tag="recr", name="rec_r")
                nc.vector.tensor_copy(state["rec"][:], c_t[:])

            def mk(ms):
                def f():
                    for m in ms:
                        pb = psum.tile([P, 512], f32, tag="mm", name="pb")
                        nc.tensor.matmul(pb[:], sel[:, m, :], state["rec"][:],
                                         start=True, stop=True)
                        nc.vector.tensor_mul(concatT[:, m, qsl],
                                             concatT[:, m, qsl], pb[:])
                return f

            return [r, mk([0, 1, 2]), mk([3, 4, 5])]

        # prologue: pair-0 q2=0 halves right after xT sts 0-3 land (they only
        # need those four s-tiles + wq0/wk0), THEN the second x half's
        # transposes — so scores(0)/exp(0) start ~25us earlier.
        proj_half(0, "q", 0)
        proj_half(0, "k", 0)
        x_quarter(2, nc.sync)
        x_quarter(3, nc.scalar)
        load_wv()
        init_consts()
        load_w(1, cast=nc.gpsimd)
        # fill the prologue's DMA-wait PE gaps with the first V-pass tiles
        v_chunk(0, range(0, 2))

        Wo_sb = None
        bo_bc = None

        def stage_wo():
            nonlocal Wo_sb, bo_bc
            bo_bc = bcast.tile([P, E], f32, tag="bbc", name="bo_bc")
            bcast_load(bo_bc[:], bo_d[:])
            # ACT is the attention-phase bottleneck: keep Wo's DMA issue off
            # the scalar queue (gpsimd SWDGE is idle here)
            Wo_sb = wbig.tile([P, KT, E], mdt, tag="wbig", name="Wo_sb")
            load_wbig(Wo_sb, Wo_d[:].rearrange("(ko p) m -> p ko m", p=P),
                      nc.gpsimd, cast=nc.gpsimd)

        # Micro-op fillers (~3 matmuls each): one is emitted after every
        # scores/attnV quarter so the PE always has independent work while
        # ACT drains the score PSUM tiles; leftovers emit after c3.
        # Deadlines: pair m's proj micros land before their consuming
        # scores; V chunks stay one attnV-chunk ahead; norm0/out st0-2
        # overlap the last two exps.
        n0 = micros_norm(0)
        fillers = {
            0: micros_proj(0, "k", 1) + micros_proj(0, "q", 1)
               + [micro_v(0, 2), micro_v(0, 3), micro_v(0, 4), micro_v(0, 5)],
            1: [micro_v(0, 6), micro_v(0, 7)]
               + micros_proj(1, "q", 0) + micros_proj(1, "k", 0),
            2: micros_proj(1, "k", 1) + micros_proj(1, "q", 1),
            3: micros_proj(2, "q", 0) + micros_proj(2, "k", 0)
               + [stage_wo, micro_v(1, 0), micro_v(1, 1)],
            4: micros_proj(2, "k", 1) + micros_proj(2, "q", 1)
               + [micro_v(1, 2), micro_v(1, 3)],
            5: micros_proj(3, "q", 0) + micros_proj(3, "k", 0)
               + [micro_v(1, 4), micro_v(1, 5)],
            6: micros_proj(3, "k", 1) + micros_proj(4, "q", 0)
               + micros_proj(4, "k", 0) + [micro_v(1, 6)],
            7: micros_proj(4, "k", 1) + micros_proj(5, "q", 0)
               + micros_proj(5, "k", 0) + [micro_v(1, 7)],
            8: micros_proj(5, "k", 1) + micros_proj(3, "q", 1),
            9: micros_proj(4, "q", 1) + micros_proj(5, "q", 1),
            10: [n0[1], n0[2]] + micros_out(0),
            11: micros_out(1) + micros_out(2) + micros_out(3),
        }
        loadw_at = {1: 2, 3: 3, 4: 4, 5: 5}
        post = {9: [n0[0]]}

        for i in range(len(units)):
            if i in loadw_at:
                load_w(loadw_at[i], cast=nc.gpsimd)
            chunks = list(fillers.get(i, []))
            prev = i - 1
            for c in range(4):
                scores_c(i, c)
                if prev >= 0:
                    attnv_chunk(prev, (2 * c, 2 * c + 1))
                if c < len(chunks):
                    chunks[c]()
            for ch in chunks[4:]:
                ch()
            if prev >= 0:
                attnv_finish(prev)
            for p_fn in post.get(i, []):
                p_fn()
        # tail: last attnV, q2=1 normalization, second output half
        for c in range(4):
            attnv_chunk(11, (2 * c, 2 * c + 1))
        attnv_finish(11)
        for f in micros_norm(1):
            f()
        for st in range(4, 8):
            for f in micros_out(st):
                f()

    _split_excess_waits(nc)
    return nc


def run_spmd(inputs, Wq, bq, Wk, bk, Wv, bv, Wo, bo,
             mm_dtype="bf16", e_dtype="bf16", crossbase=False, trace=False):
    key = (mm_dtype, e_dtype)
    if key not in _NC_CACHE:
        _NC_CACHE[key] = build(mm_dtype, e_dtype)
    nc = _NC_CACHE[key]
    x = np.asarray(inputs, dtype=np.float32)
    common = {
        "Wq": np.asarray(Wq, np.float32), "Wk": np.asarray(Wk, np.float32),
        "Wv": np.asarray(Wv, np.float32), "Wo": np.asarray(Wo, np.float32),
        "bq": np.asarray(bq, np.float32), "bk": np.asarray(bk, np.float32),
        "bv": np.asarray(bv, np.float32), "bo": np.asarray(bo, np.float32),
    }
    in_maps = [dict(common, x=np.ascontiguousarray(x[b])) for b in range(x.shape[0])]
    res = run_bass_kernel_spmd(nc, in_maps, core_ids=list(range(len(in_maps))),
                               trace=trace)
    out = np.stack([res.results[b]["out"] for b in range(len(in_maps))], axis=0)
    return out, res


def kernel(inputs, Wq, bq, Wk, bk, Wv, bv, Wo, bo):
    out, _ = run_spmd(inputs, Wq, bq, Wk, bk, Wv, bv, Wo, bo)
    return out
